# revision 137
# baseline (speedup 1.0000x reference)
"""Anchored self-attention on 8 TRN2 NeuronCores — data-parallel over batch.

Reference computation per sample (C=256 channels, N=H*W=4096 positions):
    q = Wq x + bq; k = Wk x + bk; v = Wv x + bv; anchor = Wa x + ba
    scores = q^T k   [N, N];  attn = softmax(scores, axis=-1)
    out = x + attn @ v^T (as [C,N]) + anchor

B=8 samples -> one sample per NeuronCore, no collectives.

Per-core algorithm (all layouts chosen so reductions land on the free axis
or inside the PE array):
  - host passes x in two layouts: xT [N,C] bf16 (residual) and xb [C,N] fp16
    (matmul operand), plus packed weights. fp16 runs at the same 1 cyc/row
    as bf16 on the PE but with 3 extra mantissa bits.
  - scores are factored: q^T k = x^T (Wq^T Wk) x + bias terms. M = Wq^T Wk
    and u = Wk^T bq are precomputed on the host into the weight pack;
    z = M^T x replaces both q and k projections (half the GEMM work).
    Score terms constant across keys drop by softmax invariance; the
    surviving per-key term t[m] = u.x_m folds into the exp bias.
  - vT is augmented with a ones column -> the attended matmul's PSUM
    accumulates softmax row-sums in column 256 for free.
  - scoresT tile [m=128, n=512] = x_chunk^T z_chunk (PSUM f32), then ACT
    computes exp(scores + t[m] - 104) straight out of PSUM into bf16 SBUF.
    The fixed shift replaces the row-max subtraction (scores here are
    bounded well under 104+88, and terms >90 below the row max underflow
    to 0 harmlessly), which would otherwise be a partition-axis reduction.
  - attendedT[n-tile] [128, 257] accumulates over all 32 key tiles in PSUM.
  - at_sb folds anchor^T + (x^T + ba) (the +ba is host-side, in xT) so the
    epilogue is two DVE ops: scale by 1/rowsum (col 256), add at_sb, DMA.
Output is outT [N, C] f32 per core; host transposes back.

Scheduling notes (cost-model driven; PE roofline for the fp16 matmul work
is ~240us, this schedule models ~248us = ~97% PE occupancy):
  - PE p-states: full clock only after ~3us of continuous busy. A chain of
    throwaway warm-up matmuls on memset data keeps the PE busy from ~0.9us
    (while input DMAs stream in), so real work starts at full 2.4GHz.
  - Setup (z, v+anchor, tsh) is interleaved in four batches of m-tiles so
    PE consumption stays behind DMA supply, and PSUM eviction work is
    balanced across DVE (vt adds) and ACT (z + most anchor evictions) —
    GpSimd cannot touch PSUM on this target. z evictions defer to the next
    batch (they only gate psA ring reuse), and every eviction ring is laid
    out so slot reuse trails its reader by >2us.
  - Attention groups use a 2-deep exp->attended pipeline carried ACROSS
    group boundaries: attended(mt-2) is emitted after scores(mt), so the
    scores->ACT-exp->attended chain (~870ns) hides under two PE iterations
    (~1.7us) and group handoffs have no flush bubble. Group 0's first four
    score chunks are hoisted into the setup tail.
  - Last group keeps the all-exps-first structure; its final output tile is
    split into two half-width chains, and the closing half folds at*rowsum
    into PSUM via an identity matmul so the very last epilogue is a single
    ACT multiply by 1/rowsum -> DMA (shortest possible post-matmul tail).
"""

import numpy as np
import ml_dtypes

import concourse.tile as tile
from concourse import bacc, mybir
from concourse.bass_utils import run_bass_kernel_spmd

B, C, HH, WW = 8, 256, 64, 64
N = HH * WW          # 4096 spatial positions
P = 128              # partitions
NT = N // P          # 32 tiles of 128 along n/m
NG = 8               # n groups
GW = N // NG         # 512 = group width (one PSUM bank of f32)
CA = C + 1           # 257: v augmented with ones column
NQ = 4               # quarters of the n axis for setup interleave
QW = N // NQ         # 1024
SHIFT = -104.0       # exp(score + SHIFT); max observed score ~130 < 104+88
NWARM = 31           # warm-up matmuls (128-free) until xb q0 lands
NWARM2 = 0           # optional filler warm-ups between m_t and z
QSPLIT = [0, 10, 17, 25, 32]    # setup batches of m-tiles
AN_DVE = [3, 3, 3, 4]          # per batch: last k anchors evict on DVE
                               # (last batch: ACT is full with z + exps)

F32 = mybir.dt.float32
BF16 = mybir.dt.bfloat16
FP16 = mybir.dt.float16
BF = ml_dtypes.bfloat16

# wpack column layout (fp16), DMA'd in 3 pieces ordered by first use:
#   W  [0:514)      M row-chunks (2x256) | u cols (2)
#   P3a[514:898)    bv (256) | ones (128)
#   P3b[898:1922)   [wv0T|wa0T] (512) | [wv1T|wa1T] (512)
# M = Wq^T Wk and u = Wk^T bq are precomputed on the HOST (identical
# precision: the on-device path also rounded the f32 PSUM product to
# fp16), which removes the m_t matmul+eviction chain from the critical
# startup path — the first z chunk starts as soon as M and xb land.
# v and anchor share one 512-wide PSUM tile per m-tile, so their weight
# chunks are packed side by side; the softmax ones-column is memset into
# vt_sb instead of riding the v matmul.
O_U = 512
O_P3A = 514
O_BV = 514
O_ONES = O_BV + C          # 770
O_WVWA = 898               # 2 chunks of 512
WPACK = O_WVWA + 2 * GW    # 1922

_CACHE = {}
LAST_RESULT = None


def _build():
    nc = bacc.Bacc("TRN2", target_bir_lowering=False, debug=False, num_devices=8)

    xT_d = nc.dram_tensor("xT", [N, C], BF16, kind="ExternalInput").ap()
    xb_d = nc.dram_tensor("xb", [C, N], FP16, kind="ExternalInput").ap()
    wp_d = nc.dram_tensor("wp", [P, WPACK], FP16, kind="ExternalInput").ap()
    id_d = nc.dram_tensor("ident", [P, P], BF16, kind="ExternalInput").ap()
    out_d = nc.dram_tensor("out", [N, C], F32, kind="ExternalOutput").ap()

    Exp = mybir.ActivationFunctionType.Exp
    Ident = mybir.ActivationFunctionType.Identity

    with tile.TileContext(nc) as tc:
        with (
            tc.tile_pool(name="const", bufs=1) as cpool,
            tc.tile_pool(name="big", bufs=1) as bpool,
            tc.tile_pool(name="et", bufs=34) as epool,
            tc.tile_pool(name="ot", bufs=4) as opool,
            tc.tile_pool(name="psS", bufs=4, space="PSUM") as psS,
            tc.tile_pool(name="psA", bufs=4, space="PSUM") as psA,
        ):
            # ---- PE warm-up: memset garbage, matmul it while DMAs stream.
            # Keeps the PE continuously busy from ~0.8us so the p-state is
            # fully ramped (2.4GHz) when the real weights land (~3.3us).
            warm_t = cpool.tile([P, P], FP16, tag="warm", name="warm")
            nc.gpsimd.memset(warm_t[:], 0.0)
            # weights first: m_t gates everything
            wp_t = cpool.tile([P, WPACK], FP16, tag="wp", name="wp")
            for _ in range(NWARM):
                wps = psS.tile([P, GW], F32, tag="s", name="wps")
                nc.tensor.matmul(wps[:, 0:P], warm_t[:], warm_t[:],
                                 start=True, stop=True)

            m_t = [wp_t[:, j * C:(j + 1) * C] for j in range(2)]  # M rows
            u_sb = wp_t[:, O_U:O_U + 2]
            wvwa_t = [wp_t[:, O_WVWA + i * GW:O_WVWA + (i + 1) * GW] for i in range(2)]
            bv_t = wp_t[0:1, O_BV:O_BV + C]
            ones_t = wp_t[0:1, O_ONES:O_ONES + P]
            shift_t = cpool.tile([P, 1], F32, tag="shift", name="shift")
            nc.vector.memset(shift_t[:], SHIFT)
            # pre-warm ACT LUTs for Exp/Identity so the first real use
            # doesn't pay the table-load stall mid-kernel
            aw_t = cpool.tile([1, 1], F32, tag="aw", name="aw")
            nc.scalar.activation(aw_t[0:1, 0:1], shift_t[0:1, 0:1],
                                 mybir.ActivationFunctionType.Exp)
            nc.scalar.activation(aw_t[0:1, 0:1], shift_t[0:1, 0:1],
                                 mybir.ActivationFunctionType.Identity)

            # ---- activation DMAs: xb quarters paced with setup consumption;
            # the small bias piece and wv/wa before the first v tile; xT (bf16
            # residual) quarters trail one quarter behind xb.
            # both xb halves live in one [P, 2N] tile so each quarter is a
            # single DMA (the shared HWDGE generator serializes at ~650ns per
            # DMA; fewer, bigger DMAs keep the early supply gen-bound phase
            # short)
            xb_sb = bpool.tile([P, 2 * N], FP16, tag="xb", name="xb")
            xb_t = [xb_sb[:, i * N:(i + 1) * N] for i in range(2)]
            xbq_w = xb_sb[:].rearrange("p (i n) -> p i n", i=2)
            xbq_r = xb_d.rearrange("(i p) n -> p i n", p=P)
            xt_sb = bpool.tile([P, NT * C], BF16, tag="xt", name="xt")
            xt_r = xt_sb[:].rearrange("p (t c) -> p t c", c=C)
            xT_r = xT_d.rearrange("(t p) c -> p t c", p=P)
            TQ = NT // NQ  # 8 m-tiles per quarter
            # xb quarters first — they gate the setup's PE work; the xT
            # residual quarters are only read by the group-phase at-folds
            # (~25us+), so they queue after all xb
            nc.sync.dma_start(xbq_w[:, :, 0:GW], xbq_r[:, :, 0:GW])
            nc.sync.dma_start(wp_t[:, 0:O_P3A], wp_d[:, 0:O_P3A])   # M | u
            nc.sync.dma_start(xbq_w[:, :, GW:QW], xbq_r[:, :, GW:QW])
            nc.sync.dma_start(wp_t[:, O_WVWA:WPACK], wp_d[:, O_WVWA:WPACK])
            nc.sync.dma_start(wp_t[:, O_P3A:O_WVWA], wp_d[:, O_P3A:O_WVWA])
            for q in range(1, NQ):
                nc.sync.dma_start(xbq_w[:, :, q * QW:(q + 1) * QW],
                                  xbq_r[:, :, q * QW:(q + 1) * QW])
            for q in range(NQ):
                nc.sync.dma_start(xt_r[:, q * TQ:(q + 1) * TQ, :],
                                  xT_r[:, q * TQ:(q + 1) * TQ, :])
            # identity (bf16) for the tail's at-fold matmul; needed ~200us in
            id_t = cpool.tile([P, P], BF16, tag="id", name="id")
            nc.sync.dma_start(id_t[:], id_d[:])

            zb_t = [bpool.tile([P, N], FP16, tag=f"zb{i}", name=f"zb{i}") for i in range(2)]
            vt_sb = bpool.tile([P, NT * CA], BF16, tag="vt", name="vt")
            an_sb = bpool.tile([P, NT * C], BF16, tag="an", name="an")
            at_sb = bpool.tile([P, NT * C], BF16, tag="at", name="at")
            # softmax row-sum ones column of the augmented v (col 256 of each
            # m-tile), set once up front instead of riding the v matmul
            nc.vector.memset(
                vt_sb[:].rearrange("p (t c) -> p t c", c=CA)[:, :, C:C + 1], 1.0)

            # ---- scores factorization: scoresT[m,n] = x_m . z_n + u.x_m
            # (+ per-n terms that softmax drops); z = M^T x with M = Wq^T Wk
            # and u = Wk^T bq, both precomputed on the host into the pack.

            def z_chunk(nb):
                # z[:, nb*GW:(nb+1)*GW] for both o-chunks, chains interleaved.
                # z tiles ride the psA ring (every PSUM tile is a full bank)
                # so their drain never blocks a vat tile's slot; the ACT
                # evictions are emitted separately at quarter end (they only
                # gate the NEXT quarter's psA slot reuse), keeping ACT free
                # for the anchor evictions that release vat slots.
                pss = [psA.tile([P, GW], F32, tag="a", name="zps") for _ in range(2)]
                for ic in range(2):
                    nc.tensor.matmul(pss[ic][:], m_t[0][:, ic * P:(ic + 1) * P],
                                     xb_t[0][:, nb * GW:(nb + 1) * GW],
                                     start=True, stop=False)
                for ic in range(2):
                    nc.tensor.matmul(pss[ic][:], m_t[1][:, ic * P:(ic + 1) * P],
                                     xb_t[1][:, nb * GW:(nb + 1) * GW],
                                     start=False, stop=True)
                return pss

            def z_evict(nb, pss):
                for ic in range(2):
                    nc.scalar.activation(zb_t[ic][:, nb * GW:(nb + 1) * GW],
                                         pss[ic][:], Ident, bias=0.0)

            # first quarter of z before the bias broadcasts (whose weights
            # land later). Its evictions emit immediately (ACT is idle early
            # and batch 1's z chunks reuse these psA slots right away);
            # later batches defer evictions as usual.
            zpend = []
            for nb in range(2):
                z_evict(nb, z_chunk(nb))

            # bvb broadcast is emitted after the first two vat tiles (its
            # bias piece is the last DMA of the startup group); the scheduler
            # floats the dependent vt adds until it lands
            bvb_t = cpool.tile([P, C], F32, tag="bvb", name="bvb")

            def bvb_bcast():
                ps = psS.tile([P, GW], F32, tag="s", name="bvps")
                nc.tensor.matmul(ps[:, 0:C], ones_t[0:1, :], bv_t[0:1, :],
                                 start=True, stop=True)
                nc.vector.tensor_copy(bvb_t[:], ps[:, 0:C])

            tsh_sb = cpool.tile([P, NT], F32, tag="tsh", name="tsh")

            def an_on_dve(t):
                q = max(i for i in range(NQ) if QSPLIT[i] <= t)
                return QSPLIT[q + 1] - t <= AN_DVE[q]

            def vat_tile(t):
                # v and anchor for m-tile t share one [P, 512] PSUM tile
                # (cols 0:256 = v, 256:512 = anchor); tsh rides psA.
                # Per-tile engine split: PE matmuls, DVE vt-add (+bv), ACT
                # anchor copy, GpSimd z copies — no engine is double-loaded.
                ps = psS.tile([P, GW], F32, tag="s", name="vaps")
                nc.tensor.matmul(ps[:], xb_t[0][:, t * P:(t + 1) * P], wvwa_t[0][:],
                                 start=True, stop=False)
                nc.tensor.matmul(ps[:], xb_t[1][:, t * P:(t + 1) * P], wvwa_t[1][:],
                                 start=False, stop=True)
                nc.vector.tensor_add(vt_sb[:, t * CA:t * CA + C], ps[:, 0:C],
                                     bvb_t[:])
                # anchor eviction engine chosen per tile to balance DVE (vt
                # adds) against ACT (z evictions + exps); the residual fold
                # an+xt happens during the group phase.
                if an_on_dve(t):
                    nc.vector.tensor_copy(an_sb[:, t * C:(t + 1) * C],
                                          ps[:, C:2 * C])
                else:
                    nc.scalar.activation(an_sb[:, t * C:(t + 1) * C],
                                         ps[:, C:2 * C], Ident, bias=0.0)

            def tsh_burst(tiles):
                # tsh for a batch of m-tiles (tiny matmuls); detached from
                # vat_tile so the vat tiles never touch the psA ring, whose
                # slots drain at z-eviction pace
                for t in tiles:
                    tps = psA.tile([P, CA], F32, tag="a", name="tps")
                    nc.tensor.matmul(tps[:, 0:1], xb_t[0][:, t * P:(t + 1) * P],
                                     u_sb[:, 0:1], start=True, stop=False)
                    nc.tensor.matmul(tps[:, 0:1], xb_t[1][:, t * P:(t + 1) * P],
                                     u_sb[:, 1:2], start=False, stop=True)
                    nc.vector.tensor_scalar_add(tsh_sb[:, t:t + 1], tps[:, 0:1],
                                                SHIFT)

            # q0 is DMA-limited at the front (wvwa lands after xb q0), so its
            # ---- attention machinery (used from the setup tail onward) ----
            # Groups 0..6: 2-deep scores->exp->attended pipeline. attended for
            # tile mt-2 is emitted after scores for tile mt, so the ACT exp
            # latency (~870ns) hides under two PE iterations (~1.7us).
            att_ctx = {}
            pend = []

            def score_chunk(g0, mt):
                sps = psS.tile([P, GW], F32, tag="s", name="sps")
                nc.tensor.matmul(sps[:], xb_t[0][:, mt * P:(mt + 1) * P],
                                 zb_t[0][:, g0 * GW:(g0 + 1) * GW],
                                 start=True, stop=False)
                nc.tensor.matmul(sps[:], xb_t[1][:, mt * P:(mt + 1) * P],
                                 zb_t[1][:, g0 * GW:(g0 + 1) * GW],
                                 start=False, stop=True)
                et = epool.tile([P, GW], BF16, tag="e", name="e")
                nc.scalar.activation(et[:], sps[:], Exp,
                                     bias=tsh_sb[:, mt:mt + 1])
                pend.append((g0, mt, et))
                return et

            # z chunks run first; later quarters interleave z mid-quarter so
            # each GpSimd z-drain has >2us before its PSUM slot is reused.
            # The last quarter also interleaves group 0's first four score
            # chunks so the setup->attention handoff has PSUM/DVE slack
            # (attended pops for them are deferred to the group loop: popping
            # here would pin att_ctx[0]'s PSUM banks under the tsh tiles).
            bvb_bcast()
            for t in range(QSPLIT[1]):
                vat_tile(t)
            for q in range(1, NQ):
                for nb, pss in zpend:
                    z_evict(nb, pss)
                zpend = [(nb, z_chunk(nb)) for nb in (2 * q, 2 * q + 1)]
                tsh_burst(range(QSPLIT[q - 1], QSPLIT[q]))
                for t in range(QSPLIT[q], QSPLIT[q + 1]):
                    vat_tile(t)
                    if t >= NT - 4:
                        score_chunk(0, t - (NT - 4))
            for nb, pss in zpend:
                z_evict(nb, pss)
            tsh_burst(range(QSPLIT[NQ - 1], NT))

            def attended(att_ps, et, mt):
                for j in range(GW // P):
                    nc.tensor.matmul(
                        att_ps[j][:], et[:, j * P:(j + 1) * P],
                        vt_sb[:, mt * CA:(mt + 1) * CA],
                        start=(mt == 0), stop=(mt == NT - 1),
                    )

            def epilogue_dma(att_ap, nt_i, inv, c0, c1):
                # out[:, c0:c1] = att[:, c0:c1] * inv + at ; DMA the slab out
                o = opool.tile([P, C], F32, tag="o", name="o")
                nc.vector.tensor_scalar_mul(o[:, c0:c1], att_ap[:, c0:c1], inv[:])
                nc.vector.tensor_add(o[:, c0:c1], o[:, c0:c1],
                                     at_sb[:, nt_i * C + c0:nt_i * C + c1])
                nc.sync.dma_start(out_d[nt_i * P:(nt_i + 1) * P, c0:c1], o[:, c0:c1])

            # The 2-deep pipeline carries across group boundaries: the last
            # two attended tiles of group g are paired with the first two
            # scores tiles of group g+1, so no flush bubble. att_ps contexts
            # are allocated lazily (at first attended use) so PSUM slot-reuse
            # deps follow emission order.
            def flush_one():
                g0, m0, e0 = pend.pop(0)
                if g0 not in att_ctx:
                    att_ctx[g0] = [psA.tile([P, CA], F32, tag="a", name="att")
                                   for _ in range(GW // P)]
                attended(att_ctx[g0], e0, m0)
                if m0 == NT - 1:
                    for j in range(GW // P):
                        nt_i = g0 * (GW // P) + j
                        inv = opool.tile([P, 1], F32, tag="inv", name="inv")
                        nc.vector.reciprocal(inv[:], att_ctx[g0][j][:, C:C + 1])
                        epilogue_dma(att_ctx[g0][j][:], nt_i, inv, 0, C)
                    del att_ctx[g0]

            def at_folds(g0):
                # residual fold for group g0's four output tiles, on DVE well
                # ahead of the epilogue that reads them
                for t in range(g0 * (GW // P), (g0 + 1) * (GW // P)):
                    nc.vector.tensor_add(at_sb[:, t * C:(t + 1) * C],
                                         an_sb[:, t * C:(t + 1) * C],
                                         xt_sb[:, t * C:(t + 1) * C])

            for g in range(NG - 1):
                for mt in range(len(pend) if g == 0 else 0, NT):
                    if mt == 8:
                        # residual folds emitted mid-group: early enough for
                        # the epilogues (~20us away), late enough that the
                        # scheduler can't float them back into the setup
                        # tail where DVE is already saturated
                        at_folds(g)
                    score_chunk(g, mt)
                    while len(pend) > 2:
                        flush_one()

            # last group: all exps first, then one attended chain per output
            # tile so each epilogue + DMA overlaps the next tile's matmuls.
            # The final tile is split into two half-width chains (sum-column
            # half first) so the closing epilogue+DMA tail is halved.
            g = NG - 1
            ets = []
            for mt in range(NT):
                if mt == 8:
                    at_folds(g)
                ets.append(score_chunk(g, mt))
                pend.pop()         # g7's attended is emitted per-j below
                if pend:
                    flush_one()    # drain group NG-2's last attended tiles
            for j in range(GW // P - 1):
                att = psA.tile([P, CA], F32, tag="a", name="att")
                for mt in range(NT):
                    nc.tensor.matmul(
                        att[:], ets[mt][:, j * P:(j + 1) * P],
                        vt_sb[:, mt * CA:(mt + 1) * CA],
                        start=(mt == 0), stop=(mt == NT - 1),
                    )
                nt_i = g * (GW // P) + j
                inv = opool.tile([P, 1], F32, tag="inv", name="inv")
                nc.vector.reciprocal(inv[:], att[:, C:C + 1])
                epilogue_dma(att[:], nt_i, inv, 0, C)
            # final tile: cols [128:257) (incl. row-sum col) first, then
            # [0:128). The closing half folds at*s into PSUM via an identity
            # matmul (53ns on PE) so the very last epilogue is a single ACT
            # multiply by 1/s — the shortest possible post-matmul tail.
            j = GW // P - 1
            nt_i = g * (GW // P) + j
            CH = C // 2
            CW = C - CH
            attB = psA.tile([P, CA], F32, tag="a", name="attB")
            for mt in range(NT):
                nc.tensor.matmul(
                    attB[:, 0:CW + 1], ets[mt][:, j * P:(j + 1) * P],
                    vt_sb[:, mt * CA + CH:(mt + 1) * CA],
                    start=(mt == 0), stop=(mt == NT - 1),
                )
            inv = opool.tile([P, 1], F32, tag="inv", name="inv")
            nc.vector.reciprocal(inv[:], attB[:, CW:CW + 1])
            ssum = opool.tile([P, 1], F32, tag="ssum", name="ssum")
            nc.vector.tensor_copy(ssum[:], attB[:, CW:CW + 1])
            ats = opool.tile([P, CH], BF16, tag="ats", name="ats")
            nc.scalar.activation(ats[:], at_sb[:, nt_i * C:nt_i * C + CH],
                                 Ident, scale=ssum[:])
            oB = opool.tile([P, C], F32, tag="o", name="oB")
            nc.vector.tensor_scalar_mul(oB[:, CH:C], attB[:, 0:CW], inv[:])
            nc.vector.tensor_add(oB[:, CH:C], oB[:, CH:C],
                                 at_sb[:, nt_i * C + CH:(nt_i + 1) * C])
            nc.sync.dma_start(out_d[nt_i * P:(nt_i + 1) * P, CH:C], oB[:, CH:C])
            attA = psA.tile([P, CA], F32, tag="a", name="attA")
            for mt in range(NT):
                nc.tensor.matmul(
                    attA[:, 0:CH], ets[mt][:, j * P:(j + 1) * P],
                    vt_sb[:, mt * CA:mt * CA + CH],
                    start=(mt == 0), stop=False,
                )
            nc.tensor.matmul(attA[:, 0:CH], id_t[:, 0:P], ats[:],
                             start=False, stop=True)
            oA = opool.tile([P, C], F32, tag="o", name="oA")
            nc.vector.tensor_scalar_mul(oA[:, 0:CH], attA[:, 0:CH], inv[:])
            nc.sync.dma_start(out_d[nt_i * P:(nt_i + 1) * P, 0:CH], oA[:, 0:CH])

    nc.compile()
    return nc


def _get_nc():
    if "nc" not in _CACHE:
        nc = _build()
        # Key the NEFF cache on the BIR content: the HLO-level cache does not
        # hash the bass graph (it rides in backend_config), so two different
        # kernels with identical I/O signatures would otherwise silently
        # share one stale NEFF.
        import hashlib
        import os
        h = hashlib.sha256(nc.to_json_bytes()).hexdigest()[:16]
        os.environ["NEURON_COMPILE_CACHE_URL"] = f"/tmp/neuron-cc-cache-{h}"
        # The jax executable cache must also be BIR-keyed: its key does not
        # cover the custom_call backend_config where the BIR rides.
        os.environ["JAX_COMPILATION_CACHE_DIR"] = f"/tmp/jax-cache-{h}"
        try:
            import jax
            jax.config.update("jax_compilation_cache_dir", f"/tmp/jax-cache-{h}")
        except Exception:
            pass
        _CACHE["nc"] = nc
    return _CACHE["nc"]


def _pack_weights(Wq, bq, Wk, bk, Wv, bv, Wa, ba):
    wp = np.zeros((P, WPACK), np.float32)
    wvT, waT = Wv.T, Wa.T                          # [ci, co]
    M = Wq.T @ Wk                                  # scores kernel matrix
    u = Wk.T @ bq                                  # per-key score bias
    for i in range(2):
        r = slice(i * P, (i + 1) * P)
        wp[:, i * C:(i + 1) * C] = M[r]            # M row-chunks, lhsT-ready
        wp[:, O_U + i] = u[r]
        wp[:, O_WVWA + i * GW:O_WVWA + i * GW + C] = wvT[r]
        wp[:, O_WVWA + i * GW + C:O_WVWA + (i + 1) * GW] = waT[r]
    wp[0, O_BV:O_BV + C] = bv
    wp[0, O_ONES:O_ONES + P] = 1.0
    # bk is unused: its score contribution is constant per softmax row
    return wp.astype(np.float16)


def kernel(**inputs):
    global LAST_RESULT
    x = np.asarray(inputs["x"], dtype=np.float32)
    Wq = np.asarray(inputs["Wq"], dtype=np.float32)
    bq = np.asarray(inputs["bq"], dtype=np.float32)
    Wk = np.asarray(inputs["Wk"], dtype=np.float32)
    bk = np.asarray(inputs["bk"], dtype=np.float32)
    Wv = np.asarray(inputs["Wv"], dtype=np.float32)
    bv = np.asarray(inputs["bv"], dtype=np.float32)
    Wa = np.asarray(inputs["Wa"], dtype=np.float32)
    ba = np.asarray(inputs["ba"], dtype=np.float32)

    wp = _pack_weights(Wq, bq, Wk, bk, Wv, bv, Wa, ba)

    in_maps = []
    for b in range(B):
        xs = x[b].reshape(C, N)
        # xT carries the residual with the anchor bias pre-folded: x^T + ba
        in_maps.append({
            "xT": (xs.T + ba[None, :]).astype(BF),
            "xb": xs.astype(np.float16),
            "wp": wp,
            "ident": np.eye(P, dtype=np.float32).astype(BF),
        })

    nc = _get_nc()
    res = run_bass_kernel_spmd(nc, in_maps, core_ids=list(range(B)))
    LAST_RESULT = res

    out = np.empty((B, C, HH, WW), np.float32)
    for b in range(B):
        outT = res.results[b]["out"]          # [N, C]
        out[b] = outT.T.reshape(C, HH, WW)
    return out
